# revision 58
# baseline (speedup 1.0000x reference)
"""Causal multi-head self-attention on 8 TRN2 NeuronCores.

Problem (hardcoded): x (4, 2048, 1024) f32, W_qkv (3072, 1024), W_o (1024, 1024).
  qkv = x @ W_qkv.T; q,k,v split -> (B,H,T,DK) with H=16, DK=64
  scores = q k^T / 8 + causal mask; attn = softmax; out = (attn v) @ W_o.T

Sharding: core = 2*b + hg  (b in 0..3 batches, hg in 0..1 head-groups of 8 heads).
Each core computes a partial out[b] over its 512 attn columns; host sums pairs.

Per-core dataflow ("T-attention": t2-on-partitions S^T tiles), single fused
stream built to keep the PE continuously busy (TRN2 PE p-state drops 2.4GHz
-> 1.2GHz on any stall, so stalls cost double):
  - S^T tile [t2:128, t1<=512] = kT_h^T-slice x qT_h-slice (K=DK=64, bf16);
    causal handled by skipping fully-masked column ranges and one [128,128]
    triangle mask add on the diagonal block.
  - exp via ACT with fused 1/8 scale (no max-subtraction; |scores|/8 <~ 2).
  - AV: psum[72, 512] += v_aug^T-slice x P^T tile (rows 64..71 = row sums l).
    AV pairs are emitted AV_LAG pairs behind the S/exp stream so the PE
    never waits on the ACT exp.
  - All projection work (qkv projections for the NEXT chunk, out_proj of
    earlier chunks, normalize broadcasts) is interleaved into the S/AV slot
    stream as dependency-free PE filler, drained adaptively so the PE queue
    never runs dry mid-chunk; junk warm matmuls bridge the DMA-paced start.
  - normalize: batched reciprocal_approx_fast of l per chunk, head-pair
    partition-broadcast via a K=8 fp16 PE matmul, multiply on DVE.
  - out_proj: attn^T x W_o-slice, bf16 partials to DRAM (host sums in f32).

Notes from hardware measurement (vs the CoreSim cost model): fp8 DoubleRow
matmuls run at the SAME rate as bf16 on real TRN2 (not 0.5 cyc/row), so
compensated-fp8 projections lose; the core power-throttles to ~0.85-0.9x
under sustained full-rate bf16 matmul load, putting the realistic PE ceiling
near 2.0-2.1 cols/ns.
"""

import sys
from collections import deque

import numpy as np

sys.path.insert(0, "/opt/trn_rl_repo")

import ml_dtypes  # noqa: E402

from concourse import bacc, bass, mybir, tile  # noqa: E402
from concourse.bass_utils import run_bass_kernel_spmd  # noqa: E402

FP32 = mybir.dt.float32
FP16 = mybir.dt.float16
BF16 = mybir.dt.bfloat16

B, T, D, H, DK = 4, 2048, 1024, 16, 64
NCORES = 8
E = 512          # qkv columns per head-group
NH = 8           # heads per core
P = 128
DCH = D // P     # 8 contraction chunks for the projections
EC = E // P      # 4 e-chunks for q/k
NT512 = T // 512
NT128 = T // P

MASK_VAL = -1e9
EXP_SCALE = 0.125
AV_LAG = 3                     # AV pairs trail the S/exp stream by this many
DRAIN_BUDGET = [10, 6, 5, 4]   # max filler matmuls per slot, per chunk


def _emit(nc, tc, ctx, dr, out):
    consts = ctx.enter_context(tc.tile_pool(name="consts", bufs=1))
    persist = ctx.enter_context(tc.tile_pool(name="persist", bufs=1))

    # Persistent SBUF state
    qT = persist.tile([P, EC, T], BF16, name="qT")        # e = ec*128+p
    kT = persist.tile([P, EC, T], BF16, name="kT")
    vt = persist.tile([P, NT128, NH, DK + NH], BF16, name="vt")  # + one-hot cols
    attn = persist.tile([P, EC, T], BF16, name="attn")    # dl = s*128+p
    wo_sb = persist.tile([P, EC, D], BF16, name="wo_sb")  # dl = s*128+p
    xT_sb = persist.tile([P, DCH, T], BF16, name="xT_sb")
    wq_sb = persist.tile([P, DCH, E], BF16, name="wq_sb")
    wk_sb = persist.tile([P, DCH, E], BF16, name="wk_sb")
    wv_sb = persist.tile([P, DCH, E], BF16, name="wv_sb")
    mtri_sb = consts.tile([P, P], FP32, name="mtri_sb")
    sel_sb = consts.tile([NH, NH * DK], FP16, name="sel_sb")

    nc.sync.dma_start(mtri_sb[:], dr["mtri"][:])
    nc.sync.dma_start(sel_sb[:], dr["sel"][:])
    # k weights first (kproj runs first), then q, v; wo last
    for j in range(DCH):
        nc.sync.dma_start(xT_sb[:, j], dr["xT"][j * P : (j + 1) * P, :])
        nc.sync.dma_start(wk_sb[:, j], dr["wk"][j * P : (j + 1) * P, :])
    for j in range(DCH):
        nc.sync.dma_start(wq_sb[:, j], dr["wq"][j * P : (j + 1) * P, :])
    for j in range(DCH):
        nc.sync.dma_start(wv_sb[:, j], dr["wv"][j * P : (j + 1) * P, :])
    for s in range(EC):
        nc.sync.dma_start(wo_sb[:, s], dr["wo"][s * P : (s + 1) * P, :])

    # head h's ones column sits at DK+h so its denominators land on a
    # distinct psum partition (64+h); other heads' columns there are zero
    nc.vector.memset(vt[:, :, :, DK:], 0.0)
    for hh in range(NH):
        nc.vector.memset(vt[:, :, hh, DK + hh], 1.0)

    pt_pool = ctx.enter_context(tc.tile_pool(name="pt", bufs=AV_LAG + 2))
    lrp = ctx.enter_context(tc.tile_pool(name="lrp", bufs=8))
    obuf = ctx.enter_context(tc.tile_pool(name="obuf", bufs=2))
    ps_s_pool = ctx.enter_context(tc.tile_pool(name="ps_s", bufs=2, space="PSUM"))
    ps_av_pool = ctx.enter_context(tc.tile_pool(name="ps_av", bufs=2, space="PSUM"))
    # shared ring for qkv-proj / out_proj / normalize-broadcast psums
    ps_misc = ctx.enter_context(tc.tile_pool(name="ps_misc", bufs=2, space="PSUM"))

    # PE p-state warmup: junk matmuls ramp the clock while input DMAs land
    warm = consts.tile([P, 512], BF16, name="warm")
    nc.vector.memset(warm[:], 0.0)
    ps_w = ps_misc.tile([P, 512], FP32, name="ps_p")

    def emit_warm(n):
        for _ in range(n):
            nc.tensor.matmul(
                ps_w[:], lhsT=warm[:, 0:P], rhs=warm[:],
                start=True, stop=True, skip_group_check=True,
            )

    # ---- filler machinery: dep-free PE work interleaved into the stream ----
    filler_q = deque()   # generator objects, FIFO; ~1 matmul per step
    steps_left = [0]

    def push_filler(gen, nsteps):
        filler_q.append(gen)
        steps_left[0] += nsteps

    def drain(n):
        for _ in range(n):
            while filler_q:
                try:
                    next(filler_q[0])
                    steps_left[0] -= 1
                    break
                except StopIteration:
                    filler_q.popleft()
            else:
                return

    def gen_qkproj(w_sb, dst, c, ecs=tuple(range(EC))):
        for ec in ecs:
            ps = ps_misc.tile([P, 512], FP32, name="ps_p")
            for j in range(DCH):
                nc.tensor.matmul(
                    ps[:],
                    lhsT=w_sb[:, j, ec * P : (ec + 1) * P],
                    rhs=xT_sb[:, j, c * 512 : (c + 1) * 512],
                    start=(j == 0),
                    stop=(j == DCH - 1),
                    skip_group_check=True,
                )
                if j < DCH - 1:
                    yield
            nc.vector.tensor_copy(dst[:, ec, c * 512 : (c + 1) * 512], ps[:])
            yield

    def gen_vproj(c):
        for t16 in range(4 * c, 4 * c + 4):
            psv = ps_misc.tile([P, NH, DK], FP32, name="ps_p")
            for j in range(DCH):
                nc.tensor.matmul(
                    psv[:],
                    lhsT=xT_sb[:, j, t16 * P : (t16 + 1) * P],
                    rhs=wv_sb[:, j, :],
                    start=(j == 0),
                    stop=(j == DCH - 1),
                    skip_group_check=True,
                )
                if j < DCH - 1:
                    yield
            nc.vector.tensor_copy(vt[:, t16, :, 0:DK], psv[:])
            yield

    def gen_outproj(c, evac_act=False):
        # evac_act: evacuate on the (idle-at-tail) ACT engine instead of the
        # DVE, shortening the final chunk's serial DVE chain
        for ti in range(4):
            t0 = c * 512 + ti * P
            for eo in range(2):
                pso = ps_misc.tile([P, 512], FP32, name="ps_p")
                for s in range(EC):
                    nc.tensor.matmul(
                        pso[:],
                        lhsT=attn[:, s, t0 : t0 + P],
                        rhs=wo_sb[:, s, eo * 512 : (eo + 1) * 512],
                        start=(s == 0),
                        stop=(s == EC - 1),
                        skip_group_check=True,
                    )
                    if s < EC - 1:
                        yield
                ob = obuf.tile([P, 512], BF16, name="ob")
                if evac_act:
                    nc.scalar.activation(
                        ob[:], pso[:], mybir.ActivationFunctionType.Copy
                    )
                else:
                    nc.vector.tensor_copy(ob[:], pso[:])
                nc.sync.dma_start(out[t0 : t0 + P, eo * 512 : (eo + 1) * 512], ob[:])
                yield

    def gen_back_pair(c, s, rec16):
        # attn[:, s, t1] *= 1/l_{2s,2s+1} (in place, both heads of the pair);
        # K=8 fp16 PE partition-broadcast + one [128,512] DVE multiply
        sl = attn[:, s, c * 512 : (c + 1) * 512]
        psb = ps_misc.tile([P, 512], FP32, name="ps_p")
        nc.tensor.matmul(
            psb[:],
            lhsT=sel_sb[:, s * P : (s + 1) * P],
            rhs=rec16,
            start=True,
            stop=True,
            skip_group_check=True,
        )
        nc.vector.tensor_tensor(sl, sl, psb[:], mybir.AluOpType.mult)
        yield

    # ---- attention stream ----
    av_tiles = {}
    cur_l = [None]  # [NH, 512] tile gathering the chunk's softmax denominators

    def finish_head(c, h, ps_av):
        po = (h % 2) * DK
        sub = h // 2
        # denominator rows (head h on psum partition DK+h, zeros elsewhere)
        # accumulate into lall BEFORE the attn evacuation: on the in-order
        # DVE queue this lets the last head's reciprocal (which gates the
        # whole normalize/out_proj tail) start one op earlier
        if h == 0:
            cur_l[0] = lrp.tile([NH, 512], FP32, name="lall")
            nc.vector.tensor_copy(cur_l[0][:], ps_av[DK : DK + NH, :])
        else:
            nc.vector.tensor_tensor(
                cur_l[0][:], cur_l[0][:], ps_av[DK : DK + NH, :], mybir.AluOpType.add
            )
        if h == NH - 1:
            rec32 = lrp.tile([NH, 512], FP32, name="rec32")
            nc.vector.reciprocal_approx_fast(out=rec32[:], in_=cur_l[0][:])
            rec16 = lrp.tile([NH, 512], FP16, name="rec16")
            nc.vector.tensor_copy(rec16[:], rec32[:])
        # evacuate psum: unnormalized AV -> attn (bf16); emitted after the
        # recip chain but still before this head's normalize multiply
        nc.vector.tensor_copy(
            attn[po : po + DK, sub, c * 512 : (c + 1) * 512], ps_av[0:DK, :]
        )
        if h == NH - 1:
            for s in range(EC):
                push_filler(gen_back_pair(c, s, rec16[:]), 1)
            if c >= 2:
                # earlier chunks' out_proj is scheduled at chunk starts
                push_filler(gen_outproj(c, evac_act=(c == NT512 - 1)), 32)

    def emit_front(c, h, jp, njt):
        po = (h % 2) * DK
        sub = h // 2
        pair = (2 * jp, 2 * jp + 1)
        ps2 = ps_s_pool.tile([P, 1024], FP32, name="ps_s")
        pt2 = pt_pool.tile([P, 1024], BF16, name="pt")
        # the two S tiles are packed back-to-back in psum (no gap even when
        # the diagonal shortens the first), so ONE exp always covers the pair
        base = 0
        bases = []
        for sl, j in enumerate(pair):
            m = j - 4 * c
            off = 0 if m < 0 else m * P
            t1lo = c * 512 + off
            bases.append(base)
            nc.tensor.matmul(
                ps2[:, base : base + 512 - off],
                lhsT=kT[po : po + DK, sub, j * P : (j + 1) * P],
                rhs=qT[po : po + DK, sub, t1lo : (c + 1) * 512],
                start=True,
                stop=True,
                skip_group_check=True,
            )
            if m >= 0:
                nc.vector.tensor_tensor(
                    ps2[:, base : base + P],
                    ps2[:, base : base + P],
                    mtri_sb[:],
                    mybir.AluOpType.add,
                )
            base += 512 - off
        nc.scalar.activation(
            pt2[:, 0:base], ps2[:, 0:base],
            mybir.ActivationFunctionType.Exp, scale=EXP_SCALE,
        )
        return pt2, bases

    def emit_av(c, h, jp, njt, pt2, bases):
        if jp == 0:
            av_tiles[(c, h)] = ps_av_pool.tile([DK + NH, 512], FP32, name="ps_av")
        ps_av = av_tiles[(c, h)]
        for sl, j in enumerate((2 * jp, 2 * jp + 1)):
            m = j - 4 * c
            off = 0 if m < 0 else m * P
            base = bases[sl]
            nc.tensor.matmul(
                ps_av[:, off:512],
                lhsT=vt[:, j, h, :],
                rhs=pt2[:, base : base + 512 - off],
                start=(j == 0),
                stop=(j == njt - 1),
                skip_group_check=True,
            )
        if jp == njt // 2 - 1:
            finish_head(c, h, av_tiles.pop((c, h)))

    # ---- prologue: only what head 0 of chunk 0 needs, emitted densely;
    # the rest of chunk 0's projections drain as early fillers so the
    # S/exp/AV stream starts as soon as the first DMA chunks land.
    # The first kproj group is paced by the xT/wk DMA stream (~1.8us per
    # contraction chunk), so junk warm matmuls fill the gaps between its
    # steps to keep the PE p-state ramped. ----
    emit_warm(3)
    for i, _ in enumerate(gen_qkproj(wk_sb, kT, 0)):
        if i < 8:
            emit_warm(6)
    for g in (
        gen_qkproj(wq_sb, qT, 0, ecs=(0,)),
        gen_vproj(0),
    ):
        for _ in g:
            pass

    # ---- main loop: S/exp/AV slots with interleaved filler ----
    av_q = deque()
    drained_total = [0]
    deadline = [0]  # fillers that must drain before the next chunk starts

    def drain_tracked(n):
        before = steps_left[0]
        drain(n)
        drained_total[0] += before - steps_left[0]

    for c in range(NT512):
        if c == 0:
            push_filler(gen_qkproj(wq_sb, qT, 0, ecs=(1, 2, 3)), 24)
        if c < NT512 - 1:
            push_filler(gen_qkproj(wk_sb, kT, c + 1), 32)
            push_filler(gen_qkproj(wq_sb, qT, c + 1), 32)
            push_filler(gen_vproj(c + 1), 32)
            deadline[0] = drained_total[0] + steps_left[0]
        if c == 3:
            push_filler(gen_outproj(0), 32)
            push_filler(gen_outproj(1), 32)
        njt = 4 * (c + 1)
        n_slots = NH * (njt // 2)
        slot = 0
        for h in range(NH):
            for jp in range(njt // 2):
                pt2, bases = emit_front(c, h, jp, njt)
                av_q.append((c, h, jp, njt, pt2, bases))
                if len(av_q) > AV_LAG:
                    emit_av(*av_q.popleft())
                # spread the queued filler evenly over this chunk's slots;
                # in the last chunk hold back a few steps so real work (not
                # a PE idle) covers the tail's reciprocal latency
                left = n_slots - slot
                avail = steps_left[0] - (8 if c == NT512 - 1 else 0)
                n = -(-avail // left) if left > 0 and avail > 0 else 0
                drain_tracked(min(n, DRAIN_BUDGET[c]))
                slot += 1
        # make sure next chunk's projections finished draining
        while drained_total[0] < deadline[0]:
            drain_tracked(4)

    while av_q:
        emit_av(*av_q.popleft())
    while filler_q:
        drain_tracked(8)


def _build_nc():
    from contextlib import ExitStack

    nc = bacc.Bacc("TRN2", target_bir_lowering=False, debug=False)
    dr = {}
    dr["xT"] = nc.dram_tensor("xT", [D, T], BF16, kind="ExternalInput")
    for nm in ("wq", "wk", "wv"):
        dr[nm] = nc.dram_tensor(nm, [D, E], BF16, kind="ExternalInput")
    dr["wo"] = nc.dram_tensor("wo", [E, D], BF16, kind="ExternalInput")
    dr["mtri"] = nc.dram_tensor("mtri", [P, P], FP32, kind="ExternalInput")
    dr["sel"] = nc.dram_tensor("sel", [NH, NH * DK], FP16, kind="ExternalInput")
    out = nc.dram_tensor("out", [T, D], BF16, kind="ExternalOutput")

    with (
        tile.TileContext(nc) as tc,
        nc.allow_low_precision(reason="fp8/f16/bf16 intermediates by design"),
        ExitStack() as ctx,
    ):
        _emit(nc, tc, ctx, {k: v[:] for k, v in dr.items()}, out[:])
    nc.compile()
    return nc


def _host_inputs(x, W_qkv, W_o):
    x = np.asarray(x, dtype=np.float32)
    W_qkv = np.asarray(W_qkv, dtype=np.float32)
    W_o = np.asarray(W_o, dtype=np.float32)
    mtri = np.tril(np.full((P, P), MASK_VAL, dtype=np.float32), -1)
    sel = np.zeros((NH, NH * DK), dtype=np.float16)
    for hh in range(NH):
        sel[hh, hh * DK : (hh + 1) * DK] = 1.0
    bf = ml_dtypes.bfloat16
    in_maps = []
    for b in range(B):
        xTb = np.ascontiguousarray(x[b].T.astype(bf))
        for hg in range(2):
            sl = slice(E * hg, E * hg + E)
            m = {"xT": xTb, "mtri": mtri, "sel": sel}
            for i, nm in enumerate(("wq", "wk", "wv")):
                m[nm] = np.ascontiguousarray(W_qkv[i * D :][sl].T.astype(bf))
            m["wo"] = np.ascontiguousarray(W_o[:, sl].T.astype(bf))
            in_maps.append(m)
    return in_maps


def _run(x, W_qkv, W_o, trace=False, tmpdir=None):
    nc = _build_nc()
    in_maps = _host_inputs(x, W_qkv, W_o)
    res = run_bass_kernel_spmd(
        nc, in_maps, list(range(NCORES)), trace=trace, tmpdir=tmpdir
    )
    out = np.empty((B, T, D), dtype=np.float32)
    for b in range(B):
        out[b] = res.results[2 * b]["out"].astype(np.float32) + res.results[
            2 * b + 1
        ]["out"].astype(np.float32)
    return out, res.exec_time_ns


def kernel(x, W_qkv, W_o):
    out, _ = _run(x, W_qkv, W_o, trace=False)
    return out


# revision 59
# speedup vs baseline: 1.0167x; 1.0167x over previous
"""Causal multi-head self-attention on 8 TRN2 NeuronCores.

Problem (hardcoded): x (4, 2048, 1024) f32, W_qkv (3072, 1024), W_o (1024, 1024).
  qkv = x @ W_qkv.T; q,k,v split -> (B,H,T,DK) with H=16, DK=64
  scores = q k^T / 8 + causal mask; attn = softmax; out = (attn v) @ W_o.T

Sharding: core = 2*b + hg  (b in 0..3 batches, hg in 0..1 head-groups of 8 heads).
Each core computes a partial out[b] over its 512 attn columns; host sums pairs.

Per-core dataflow ("T-attention": t2-on-partitions S^T tiles), single fused
stream built to keep the PE continuously busy (TRN2 PE p-state drops 2.4GHz
-> 1.2GHz on any stall, so stalls cost double):
  - S^T tile [t2:128, t1<=512] = kT_h^T-slice x qT_h-slice (K=DK=64, bf16);
    causal handled by skipping fully-masked column ranges and one [128,128]
    triangle mask add on the diagonal block.
  - exp via ACT with fused 1/8 scale (no max-subtraction; |scores|/8 <~ 2).
  - AV: psum[72, 512] += v_aug^T-slice x P^T tile (rows 64..71 = row sums l).
    AV pairs are emitted AV_LAG pairs behind the S/exp stream so the PE
    never waits on the ACT exp.
  - All projection work (qkv projections for the NEXT chunk, out_proj of
    earlier chunks, normalize broadcasts) is interleaved into the S/AV slot
    stream as dependency-free PE filler, drained adaptively so the PE queue
    never runs dry mid-chunk; junk warm matmuls bridge the DMA-paced start.
  - normalize: batched reciprocal_approx_fast of l per chunk, head-pair
    partition-broadcast via a K=8 fp16 PE matmul, multiply on DVE.
  - out_proj: attn^T x W_o-slice, bf16 partials to DRAM (host sums in f32).

Notes from hardware measurement (vs the CoreSim cost model): fp8 DoubleRow
matmuls run at the SAME rate as bf16 on real TRN2 (not 0.5 cyc/row), so
compensated-fp8 projections lose; the core power-throttles to ~0.85-0.9x
under sustained full-rate bf16 matmul load, putting the realistic PE ceiling
near 2.0-2.1 cols/ns.
"""

import sys
from collections import deque

import numpy as np

sys.path.insert(0, "/opt/trn_rl_repo")

import ml_dtypes  # noqa: E402

from concourse import bacc, bass, mybir, tile  # noqa: E402
from concourse.bass_utils import run_bass_kernel_spmd  # noqa: E402

FP32 = mybir.dt.float32
FP16 = mybir.dt.float16
BF16 = mybir.dt.bfloat16

B, T, D, H, DK = 4, 2048, 1024, 16, 64
NCORES = 8
E = 512          # qkv columns per head-group
NH = 8           # heads per core
P = 128
DCH = D // P     # 8 contraction chunks for the projections
EC = E // P      # 4 e-chunks for q/k
NT512 = T // 512
NT128 = T // P

MASK_VAL = -1e9
EXP_SCALE = 0.125
AV_LAG = 4                     # AV pairs trail the S/exp stream by this many
DRAIN_BUDGET = [10, 6, 5, 4]   # max filler matmuls per slot, per chunk


def _emit(nc, tc, ctx, dr, out):
    consts = ctx.enter_context(tc.tile_pool(name="consts", bufs=1))
    persist = ctx.enter_context(tc.tile_pool(name="persist", bufs=1))

    # Persistent SBUF state
    qT = persist.tile([P, EC, T], BF16, name="qT")        # e = ec*128+p
    kT = persist.tile([P, EC, T], BF16, name="kT")
    vt = persist.tile([P, NT128, NH, DK + NH], BF16, name="vt")  # + one-hot cols
    attn = persist.tile([P, EC, T], BF16, name="attn")    # dl = s*128+p
    wo_sb = persist.tile([P, EC, D], BF16, name="wo_sb")  # dl = s*128+p
    xT_sb = persist.tile([P, DCH, T], BF16, name="xT_sb")
    wq_sb = persist.tile([P, DCH, E], BF16, name="wq_sb")
    wk_sb = persist.tile([P, DCH, E], BF16, name="wk_sb")
    wv_sb = persist.tile([P, DCH, E], BF16, name="wv_sb")
    mtri_sb = consts.tile([P, P], FP32, name="mtri_sb")
    sel_sb = consts.tile([NH, NH * DK], FP16, name="sel_sb")

    nc.sync.dma_start(mtri_sb[:], dr["mtri"][:])
    nc.sync.dma_start(sel_sb[:], dr["sel"][:])
    # k weights first (kproj runs first), then q, v; wo last
    for j in range(DCH):
        nc.sync.dma_start(xT_sb[:, j], dr["xT"][j * P : (j + 1) * P, :])
        nc.sync.dma_start(wk_sb[:, j], dr["wk"][j * P : (j + 1) * P, :])
    for j in range(DCH):
        nc.sync.dma_start(wq_sb[:, j], dr["wq"][j * P : (j + 1) * P, :])
    for j in range(DCH):
        nc.sync.dma_start(wv_sb[:, j], dr["wv"][j * P : (j + 1) * P, :])
    for s in range(EC):
        nc.sync.dma_start(wo_sb[:, s], dr["wo"][s * P : (s + 1) * P, :])

    # head h's ones column sits at DK+h so its denominators land on a
    # distinct psum partition (64+h); other heads' columns there are zero
    nc.vector.memset(vt[:, :, :, DK:], 0.0)
    for hh in range(NH):
        nc.vector.memset(vt[:, :, hh, DK + hh], 1.0)

    pt_pool = ctx.enter_context(tc.tile_pool(name="pt", bufs=AV_LAG + 2))
    lrp = ctx.enter_context(tc.tile_pool(name="lrp", bufs=8))
    obuf = ctx.enter_context(tc.tile_pool(name="obuf", bufs=2))
    ps_s_pool = ctx.enter_context(tc.tile_pool(name="ps_s", bufs=2, space="PSUM"))
    ps_av_pool = ctx.enter_context(tc.tile_pool(name="ps_av", bufs=2, space="PSUM"))
    # shared ring for qkv-proj / out_proj / normalize-broadcast psums
    ps_misc = ctx.enter_context(tc.tile_pool(name="ps_misc", bufs=2, space="PSUM"))

    # PE p-state warmup: junk matmuls ramp the clock while input DMAs land
    warm = consts.tile([P, 512], BF16, name="warm")
    nc.vector.memset(warm[:], 0.0)
    ps_w = ps_misc.tile([P, 512], FP32, name="ps_p")

    def emit_warm(n):
        for _ in range(n):
            nc.tensor.matmul(
                ps_w[:], lhsT=warm[:, 0:P], rhs=warm[:],
                start=True, stop=True, skip_group_check=True,
            )

    # ---- filler machinery: dep-free PE work interleaved into the stream ----
    filler_q = deque()   # generator objects, FIFO; ~1 matmul per step
    steps_left = [0]

    def push_filler(gen, nsteps):
        filler_q.append(gen)
        steps_left[0] += nsteps

    def drain(n):
        for _ in range(n):
            while filler_q:
                try:
                    next(filler_q[0])
                    steps_left[0] -= 1
                    break
                except StopIteration:
                    filler_q.popleft()
            else:
                return

    def gen_qkproj(w_sb, dst, c, ecs=tuple(range(EC))):
        for ec in ecs:
            ps = ps_misc.tile([P, 512], FP32, name="ps_p")
            for j in range(DCH):
                nc.tensor.matmul(
                    ps[:],
                    lhsT=w_sb[:, j, ec * P : (ec + 1) * P],
                    rhs=xT_sb[:, j, c * 512 : (c + 1) * 512],
                    start=(j == 0),
                    stop=(j == DCH - 1),
                    skip_group_check=True,
                )
                if j < DCH - 1:
                    yield
            nc.vector.tensor_copy(dst[:, ec, c * 512 : (c + 1) * 512], ps[:])
            yield

    def gen_vproj(c):
        for t16 in range(4 * c, 4 * c + 4):
            psv = ps_misc.tile([P, NH, DK], FP32, name="ps_p")
            for j in range(DCH):
                nc.tensor.matmul(
                    psv[:],
                    lhsT=xT_sb[:, j, t16 * P : (t16 + 1) * P],
                    rhs=wv_sb[:, j, :],
                    start=(j == 0),
                    stop=(j == DCH - 1),
                    skip_group_check=True,
                )
                if j < DCH - 1:
                    yield
            nc.vector.tensor_copy(vt[:, t16, :, 0:DK], psv[:])
            yield

    def gen_outproj(c, evac_act=False):
        # evac_act: evacuate on the (idle-at-tail) ACT engine instead of the
        # DVE, shortening the final chunk's serial DVE chain
        for ti in range(4):
            t0 = c * 512 + ti * P
            for eo in range(2):
                pso = ps_misc.tile([P, 512], FP32, name="ps_p")
                for s in range(EC):
                    nc.tensor.matmul(
                        pso[:],
                        lhsT=attn[:, s, t0 : t0 + P],
                        rhs=wo_sb[:, s, eo * 512 : (eo + 1) * 512],
                        start=(s == 0),
                        stop=(s == EC - 1),
                        skip_group_check=True,
                    )
                    if s < EC - 1:
                        yield
                ob = obuf.tile([P, 512], BF16, name="ob")
                if evac_act:
                    nc.scalar.activation(
                        ob[:], pso[:], mybir.ActivationFunctionType.Copy
                    )
                else:
                    nc.vector.tensor_copy(ob[:], pso[:])
                nc.sync.dma_start(out[t0 : t0 + P, eo * 512 : (eo + 1) * 512], ob[:])
                yield

    def gen_back_pair(c, s, rec16):
        # attn[:, s, t1] *= 1/l_{2s,2s+1} (in place, both heads of the pair);
        # K=8 fp16 PE partition-broadcast + one [128,512] DVE multiply
        sl = attn[:, s, c * 512 : (c + 1) * 512]
        psb = ps_misc.tile([P, 512], FP32, name="ps_p")
        nc.tensor.matmul(
            psb[:],
            lhsT=sel_sb[:, s * P : (s + 1) * P],
            rhs=rec16,
            start=True,
            stop=True,
            skip_group_check=True,
        )
        nc.vector.tensor_tensor(sl, sl, psb[:], mybir.AluOpType.mult)
        yield

    # ---- attention stream ----
    av_tiles = {}
    cur_l = [None]  # [NH, 512] tile gathering the chunk's softmax denominators

    def finish_head(c, h, ps_av):
        po = (h % 2) * DK
        sub = h // 2
        # denominator rows (head h on psum partition DK+h, zeros elsewhere)
        # accumulate into lall BEFORE the attn evacuation: on the in-order
        # DVE queue this lets the last head's reciprocal (which gates the
        # whole normalize/out_proj tail) start one op earlier
        if h == 0:
            cur_l[0] = lrp.tile([NH, 512], FP32, name="lall")
            nc.vector.tensor_copy(cur_l[0][:], ps_av[DK : DK + NH, :])
        else:
            nc.vector.tensor_tensor(
                cur_l[0][:], cur_l[0][:], ps_av[DK : DK + NH, :], mybir.AluOpType.add
            )
        if h == NH - 1:
            rec32 = lrp.tile([NH, 512], FP32, name="rec32")
            nc.vector.reciprocal_approx_fast(out=rec32[:], in_=cur_l[0][:])
            rec16 = lrp.tile([NH, 512], FP16, name="rec16")
            nc.vector.tensor_copy(rec16[:], rec32[:])
        # evacuate psum: unnormalized AV -> attn (bf16); emitted after the
        # recip chain but still before this head's normalize multiply
        nc.vector.tensor_copy(
            attn[po : po + DK, sub, c * 512 : (c + 1) * 512], ps_av[0:DK, :]
        )
        if h == NH - 1:
            for s in range(EC):
                push_filler(gen_back_pair(c, s, rec16[:]), 1)
            if c >= 2:
                # earlier chunks' out_proj is scheduled at chunk starts
                push_filler(gen_outproj(c, evac_act=(c == NT512 - 1)), 32)

    def emit_front(c, h, jp, njt):
        po = (h % 2) * DK
        sub = h // 2
        pair = (2 * jp, 2 * jp + 1)
        ps2 = ps_s_pool.tile([P, 1024], FP32, name="ps_s")
        pt2 = pt_pool.tile([P, 1024], BF16, name="pt")
        # the two S tiles are packed back-to-back in psum (no gap even when
        # the diagonal shortens the first), so ONE exp always covers the pair
        base = 0
        bases = []
        for sl, j in enumerate(pair):
            m = j - 4 * c
            off = 0 if m < 0 else m * P
            t1lo = c * 512 + off
            bases.append(base)
            nc.tensor.matmul(
                ps2[:, base : base + 512 - off],
                lhsT=kT[po : po + DK, sub, j * P : (j + 1) * P],
                rhs=qT[po : po + DK, sub, t1lo : (c + 1) * 512],
                start=True,
                stop=True,
                skip_group_check=True,
            )
            if m >= 0:
                nc.vector.tensor_tensor(
                    ps2[:, base : base + P],
                    ps2[:, base : base + P],
                    mtri_sb[:],
                    mybir.AluOpType.add,
                )
            base += 512 - off
        nc.scalar.activation(
            pt2[:, 0:base], ps2[:, 0:base],
            mybir.ActivationFunctionType.Exp, scale=EXP_SCALE,
        )
        return pt2, bases

    def emit_av(c, h, jp, njt, pt2, bases):
        if jp == 0:
            av_tiles[(c, h)] = ps_av_pool.tile([DK + NH, 512], FP32, name="ps_av")
        ps_av = av_tiles[(c, h)]
        for sl, j in enumerate((2 * jp, 2 * jp + 1)):
            m = j - 4 * c
            off = 0 if m < 0 else m * P
            base = bases[sl]
            nc.tensor.matmul(
                ps_av[:, off:512],
                lhsT=vt[:, j, h, :],
                rhs=pt2[:, base : base + 512 - off],
                start=(j == 0),
                stop=(j == njt - 1),
                skip_group_check=True,
            )
        if jp == njt // 2 - 1:
            finish_head(c, h, av_tiles.pop((c, h)))

    # ---- prologue: only what head 0 of chunk 0 needs, emitted densely;
    # the rest of chunk 0's projections drain as early fillers so the
    # S/exp/AV stream starts as soon as the first DMA chunks land.
    # The first kproj group is paced by the xT/wk DMA stream (~1.8us per
    # contraction chunk), so junk warm matmuls fill the gaps between its
    # steps to keep the PE p-state ramped. ----
    emit_warm(3)
    for i, _ in enumerate(gen_qkproj(wk_sb, kT, 0)):
        if i < 8:
            emit_warm(6)
    for g in (
        gen_qkproj(wq_sb, qT, 0, ecs=(0,)),
        gen_vproj(0),
    ):
        for _ in g:
            pass

    # ---- main loop: S/exp/AV slots with interleaved filler ----
    av_q = deque()
    drained_total = [0]
    deadline = [0]  # fillers that must drain before the next chunk starts

    def drain_tracked(n):
        before = steps_left[0]
        drain(n)
        drained_total[0] += before - steps_left[0]

    for c in range(NT512):
        if c == 0:
            push_filler(gen_qkproj(wq_sb, qT, 0, ecs=(1, 2, 3)), 24)
        if c < NT512 - 1:
            push_filler(gen_qkproj(wk_sb, kT, c + 1), 32)
            push_filler(gen_qkproj(wq_sb, qT, c + 1), 32)
            push_filler(gen_vproj(c + 1), 32)
            deadline[0] = drained_total[0] + steps_left[0]
        if c == 3:
            push_filler(gen_outproj(0), 32)
            push_filler(gen_outproj(1), 32)
        njt = 4 * (c + 1)
        n_slots = NH * (njt // 2)
        slot = 0
        for h in range(NH):
            for jp in range(njt // 2):
                pt2, bases = emit_front(c, h, jp, njt)
                av_q.append((c, h, jp, njt, pt2, bases))
                if len(av_q) > AV_LAG:
                    emit_av(*av_q.popleft())
                # spread the queued filler evenly over this chunk's slots;
                # in the last chunk hold back a few steps so real work (not
                # a PE idle) covers the tail's reciprocal latency
                left = n_slots - slot
                avail = steps_left[0] - (8 if c == NT512 - 1 else 0)
                n = -(-avail // left) if left > 0 and avail > 0 else 0
                drain_tracked(min(n, DRAIN_BUDGET[c]))
                slot += 1
        # make sure next chunk's projections finished draining
        while drained_total[0] < deadline[0]:
            drain_tracked(4)

    while av_q:
        emit_av(*av_q.popleft())
    while filler_q:
        drain_tracked(8)


def _build_nc():
    from contextlib import ExitStack

    nc = bacc.Bacc("TRN2", target_bir_lowering=False, debug=False)
    dr = {}
    dr["xT"] = nc.dram_tensor("xT", [D, T], BF16, kind="ExternalInput")
    for nm in ("wq", "wk", "wv"):
        dr[nm] = nc.dram_tensor(nm, [D, E], BF16, kind="ExternalInput")
    dr["wo"] = nc.dram_tensor("wo", [E, D], BF16, kind="ExternalInput")
    dr["mtri"] = nc.dram_tensor("mtri", [P, P], FP32, kind="ExternalInput")
    dr["sel"] = nc.dram_tensor("sel", [NH, NH * DK], FP16, kind="ExternalInput")
    out = nc.dram_tensor("out", [T, D], BF16, kind="ExternalOutput")

    with (
        tile.TileContext(nc) as tc,
        nc.allow_low_precision(reason="fp8/f16/bf16 intermediates by design"),
        ExitStack() as ctx,
    ):
        _emit(nc, tc, ctx, {k: v[:] for k, v in dr.items()}, out[:])
    nc.compile()
    return nc


def _host_inputs(x, W_qkv, W_o):
    x = np.asarray(x, dtype=np.float32)
    W_qkv = np.asarray(W_qkv, dtype=np.float32)
    W_o = np.asarray(W_o, dtype=np.float32)
    mtri = np.tril(np.full((P, P), MASK_VAL, dtype=np.float32), -1)
    sel = np.zeros((NH, NH * DK), dtype=np.float16)
    for hh in range(NH):
        sel[hh, hh * DK : (hh + 1) * DK] = 1.0
    bf = ml_dtypes.bfloat16
    in_maps = []
    for b in range(B):
        xTb = np.ascontiguousarray(x[b].T.astype(bf))
        for hg in range(2):
            sl = slice(E * hg, E * hg + E)
            m = {"xT": xTb, "mtri": mtri, "sel": sel}
            for i, nm in enumerate(("wq", "wk", "wv")):
                m[nm] = np.ascontiguousarray(W_qkv[i * D :][sl].T.astype(bf))
            m["wo"] = np.ascontiguousarray(W_o[:, sl].T.astype(bf))
            in_maps.append(m)
    return in_maps


def _run(x, W_qkv, W_o, trace=False, tmpdir=None):
    nc = _build_nc()
    in_maps = _host_inputs(x, W_qkv, W_o)
    res = run_bass_kernel_spmd(
        nc, in_maps, list(range(NCORES)), trace=trace, tmpdir=tmpdir
    )
    out = np.empty((B, T, D), dtype=np.float32)
    for b in range(B):
        out[b] = res.results[2 * b]["out"].astype(np.float32) + res.results[
            2 * b + 1
        ]["out"].astype(np.float32)
    return out, res.exec_time_ns


def kernel(x, W_qkv, W_o):
    out, _ = _run(x, W_qkv, W_o, trace=False)
    return out
